# revision 22
# baseline (speedup 1.0000x reference)
"""Trainium2 Bass kernel for nn_CopyMechanism (optimized, v3).

Math (per batch b):
  out[g,c] = softmax_c(mask ? (score_h[g]+score_c[c]) : -inf)
             * sigmoid(gate_h[g]+gate_c[c]+b0)

softmax_c(score_h[g]+score_c[c]) == softmax_c(score_c): score_h drops out,
so copy_probs is independent of g and w_attn[:H] is unused; encoder_output
is unused by the reference. Scores are O(1): no max subtraction needed.

Layout strategy: everything is pre-blocked on the host so no on-chip
transposes are needed and all HBM traffic is fp16/bf16 (13MB/core):
  - ctx arrives as [h_p=128, jh=8, c=4096] fp16; PE matmuls with the ctx
    128x128 block *stationary* (fp16 -> FWL fast weight loads) and the
    (wg_c, wa_c) column pair *moving* put sc/gc on c-partitions directly.
  - softmax + all scalings are per-partition scalar ops; gate tiles are
    [c_p=128, g=512]: ACT sigmoid(ghb + gc bias), gh broadcast once.
  - out accumulates as bf16 [128, ci, g]; host unblocks to [G, C] f32.
Pipelining: ctx streams in decreasing-size c-chunks on two DMA queues
(hid on the second queue first); per chunk: PE dots -> DVE gc copy ->
ACT gates + Exp(masked sc) -> DVE gate*e (no Z dependency). Tail is only:
tiny Z reduce, one per-chunk *1/Z tensor_scalar, and the out-DMA stream.
ACT is the pacer in the window (32*0.71us gates); DMA in+out ~37us total.
"""
import sys

if "/opt/trn_rl_repo" not in sys.path:
    sys.path.insert(0, "/opt/trn_rl_repo")

import numpy as np
from contextlib import ExitStack

B, G, C, H = 8, 512, 4096, 1024
N_CORES = 8
P = 128
JH = H // P            # 8 h-blocks of 128
NCT = C // P           # 32 c-tiles of 128
# ctx chunk sizes in c-tiles; even chunks stream on the sync HWDGE queue,
# odd chunks (+hid) on the gpsimd SWDGE queue; the last chunk is one tile
# so the only unoverlapped gate work is minimal
SZ = [2, 4, 5, 5, 5, 5, 5, 1]
NCH = len(SZ)
CB = [0]
for s in SZ:
    CB.append(CB[-1] + s)
assert CB[-1] == NCT

_cache = {}


def _build():
    import concourse.bass as bass
    import concourse.tile as tile
    from concourse import bacc, mybir

    f32 = mybir.dt.float32
    f16 = mybir.dt.float16
    bf16 = mybir.dt.bfloat16
    i32 = mybir.dt.int32
    ADD = mybir.AluOpType.add
    MULT = mybir.AluOpType.mult

    nc = bacc.Bacc("TRN2", target_bir_lowering=False, debug=False,
                   num_devices=N_CORES)
    # ctxb[p, jh, c] = ctx[c, jh*128+p]  (fp16)
    ctx_d = nc.dram_tensor("ctx", [P, JH, C], f16, kind="ExternalInput").ap()
    # hidb[p, jh, g] = hid[g, jh*128+p]  (fp16)
    hid_d = nc.dram_tensor("hid", [P, JH, G], f16, kind="ExternalInput").ap()
    # wcols[p, jh*3+s]: s=0 wg_c, s=1 wa_c, s=2 wg_h  at h=jh*128+p  (fp16)
    w_d = nc.dram_tensor("w", [P, 3 * JH], f16, kind="ExternalInput").ap()
    # maskc[p, ci] = copy_mask[ci*128+p]
    mask_d = nc.dram_tensor("mask", [P, NCT], i32, kind="ExternalInput").ap()
    bg_d = nc.dram_tensor("bg", [1, 1], f32, kind="ExternalInput").ap()
    # outb[p, ci, g] = out[g, ci*128+p]  (bf16)
    out_d = nc.dram_tensor("out", [P, NCT, G], bf16,
                           kind="ExternalOutput").ap()

    with tile.TileContext(nc) as tc:
        with ExitStack() as ctx:
            sg = ctx.enter_context(tc.tile_pool(name="sg", bufs=1))
            ps = ctx.enter_context(
                tc.tile_pool(name="ps", bufs=1, space="PSUM"))

            # ---- PE warm-up: ~3.5us of dummy matmuls during the DMA
            # lead-in so the HAM clock is at 2.4GHz for the real work ----
            ones128 = sg.tile([P, 1], f32)
            nc.vector.memset(ones128, 1.0)
            ones_row = sg.tile([1, P], f32)
            nc.vector.memset(ones_row, 1.0)
            warm_ps = ps.tile([1, 1], f32, tag="warm")
            for _ in range(16):
                nc.tensor.matmul(warm_ps, ones128, ones128,
                                 start=True, stop=True)

            # ---- small input DMAs (gpsimd queue) ----
            wc = sg.tile([P, 3 * JH], f16)
            nc.gpsimd.dma_start(out=wc, in_=w_d)
            maskc = sg.tile([P, NCT], i32)
            nc.gpsimd.dma_start(out=maskc, in_=mask_d)
            # bg broadcast straight from DRAM via stride-0 DMA (gpsimd
            # partition_broadcast has a huge cold-start; avoid it)
            bg_b = sg.tile([P, 1], f32)
            nc.gpsimd.dma_start(
                out=bg_b,
                in_=bass.AP(tensor=bg_d.tensor, offset=bg_d.offset,
                            ap=[[0, P], [1, 1]]))

            # dummy sigmoid so the ACT table loads during the lead-in
            dummy = sg.tile([1, 1], f32)
            nc.vector.memset(dummy, 0.0)
            nc.scalar.activation(dummy, dummy,
                                 mybir.ActivationFunctionType.Sigmoid)

            # ---- big input DMAs: even ctx chunks on the sync HWDGE
            # ring; hid + odd ctx chunks on the ACT HWDGE ring (the ACT
            # engine only pays ~0.6us issue residency per DMA, spaced
            # between gate batches). gpsimd/SWDGE moves no bulk data. ----
            def ctx_chunk_dma(k):
                c0, c1 = CB[k] * P, CB[k + 1] * P
                nc.sync.dma_start(out=ctx_sb[:, :, c0:c1],
                                  in_=ctx_d[:, :, c0:c1])

            hid_sb = sg.tile([P, JH, G], f16)
            ctx_sb = sg.tile([P, JH, C], f16)
            nc.scalar.dma_start(out=hid_sb, in_=hid_d)
            for k in range(NCH):
                ctx_chunk_dma(k)

            dots = ps.tile([P, 2 * NCT], f32, tag="dots")
            gc_cols = sg.tile([P, NCT], f32)
            e_cols = sg.tile([P, NCT], f32)
            s1 = sg.tile([P, NCT], f32)
            s2 = sg.tile([P, NCT], f32)
            msc = sg.tile([P, NCT], f32)
            nc.vector.memset(msc, -30.0)
            out_sb = sg.tile([P, NCT, G], bf16)

            def chunk_dots(k):
                for ci in range(CB[k], CB[k + 1]):
                    for jh in range(JH):
                        nc.tensor.matmul(
                            dots[:, 2 * ci:2 * ci + 2],
                            ctx_sb[:, jh, ci * P:(ci + 1) * P],
                            wc[:, jh * 3:jh * 3 + 2],
                            start=(jh == 0), stop=(jh == JH - 1))

            def chunk_pre(k):
                t0, t1 = CB[k], CB[k + 1]
                # rhs col 0 = wg_c -> even dot cols are gc, odd are sc;
                # fold the gate bias b0 in here (per-partition add)
                nc.vector.tensor_scalar(
                    out=gc_cols[:, t0:t1], in0=dots[:, 2 * t0:2 * t1:2],
                    scalar1=bg_b[:, 0:1], scalar2=None, op0=ADD)
                for ci in range(t0, t1):
                    nc.scalar.activation(
                        out_sb[:, ci, :], ghb,
                        mybir.ActivationFunctionType.Sigmoid,
                        bias=gc_cols[:, ci:ci + 1])
                nc.vector.copy_predicated(
                    msc[:, t0:t1], maskc[:, t0:t1],
                    dots[:, 2 * t0 + 1:2 * t1:2])

            def pair_e(i):
                # e = exp(msc) = sig(msc)/sig(-msc) over chunk pair
                # (2i, 2i+1); same ACT table as the gates -> no reload
                a, b = CB[2 * i], CB[2 * i + 2]
                nc.scalar.activation(s1[:, a:b], msc[:, a:b],
                                     mybir.ActivationFunctionType.Sigmoid)
                nc.scalar.activation(s2[:, a:b], msc[:, a:b],
                                     mybir.ActivationFunctionType.Sigmoid,
                                     scale=-1.0)
                nc.vector.reciprocal(s2[:, a:b], s2[:, a:b])
                nc.vector.tensor_mul(e_cols[:, a:b], s1[:, a:b], s2[:, a:b])

            def chunk_ge(k):
                for ci in range(CB[k], CB[k + 1]):
                    nc.vector.tensor_scalar(
                        out=out_sb[:, ci, :], in0=out_sb[:, ci, :],
                        scalar1=e_cols[:, ci:ci + 1], scalar2=None,
                        op0=MULT)

            # ---- gh[g] = hid[g,:] @ wg_h -> broadcast to [128, G] ----
            # (hid lands first on the ACT ring, so PE does gh first)
            ghb = ps.tile([P, G], f32, tag="ghb")
            ghp = ps.tile([1, G], f32, tag="ghp")
            for jh in range(JH):
                nc.tensor.matmul(ghp, wc[:, jh * 3 + 2:jh * 3 + 3],
                                 hid_sb[:, jh, :],
                                 start=(jh == 0), stop=(jh == JH - 1))
            gh_row = sg.tile([1, G], f32)
            nc.scalar.copy(gh_row, ghp)

            # broadcast gh to all partitions with a K=1 PE matmul; the
            # gate activations read it straight from PSUM
            chunk_dots(0)
            nc.tensor.matmul(ghb, ones_row, gh_row, start=True, stop=True)

            chunk_pre(0)
            chunk_dots(1)
            chunk_pre(1)
            pair_e(0)
            chunk_ge(0)
            chunk_ge(1)
            for i in range(1, NCH // 2):
                for k in (2 * i, 2 * i + 1):
                    chunk_dots(k)
                    chunk_pre(k)
                pair_e(i)
                chunk_ge(2 * i)
                chunk_ge(2 * i + 1)

            # ---- Z = sum_c e; rz = 1/Z broadcast ----
            zred = sg.tile([P, 1], f32)
            nc.vector.reduce_sum(zred, e_cols, axis=mybir.AxisListType.X)
            zp = ps.tile([1, 1], f32, tag="zp")
            nc.tensor.matmul(zp, zred, ones128, start=True, stop=True)
            rz = sg.tile([1, 1], f32)
            nc.vector.reciprocal(rz, zp)
            # broadcast 1/Z to all partitions with a tiny PE matmul
            # (ones[1,128]^T @ rz), then stage to SBUF on DVE
            rzb_ps = ps.tile([P, 1], f32, tag="rzb")
            nc.tensor.matmul(rzb_ps, ones_row, rz, start=True, stop=True)
            rz_b = sg.tile([P, 1], f32)
            nc.vector.tensor_scalar(out=rz_b, in0=rzb_ps, scalar1=0.0,
                                    scalar2=None, op0=ADD)

            # ---- finals: one *rz per chunk, all on DVE (ACT finals are
            # 2.4us each and would pace the out stream); out-DMA stream
            # split across the sync + ACT HWDGE rings ----
            for k in range(NCH):
                t0, t1 = CB[k], CB[k + 1]
                nc.vector.tensor_scalar(
                    out=out_sb[:, t0:t1, :], in0=out_sb[:, t0:t1, :],
                    scalar1=rz_b[:, 0:1], scalar2=None, op0=MULT)
                eng = nc.sync if k % 2 == 0 else nc.scalar
                eng.dma_start(out=out_d[:, t0:t1, :],
                              in_=out_sb[:, t0:t1, :])

    nc.compile()
    return nc


def _get_nc():
    if "nc" not in _cache:
        _cache["nc"] = _build()
    return _cache["nc"]


def make_in_maps(hidden_states, context_hidden, w_attn, w_gate, b_gate,
                 copy_mask):
    # wcols[p, jh*3+s]: s=0 wg_c, s=1 wa_c, s=2 wg_h at h=jh*128+p
    w3 = np.stack([w_gate[H:], w_attn[H:], w_gate[:H]], axis=1)  # [H, 3]
    wcols = np.ascontiguousarray(
        w3.reshape(JH, P, 3).transpose(1, 0, 2).reshape(P, 3 * JH)
    ).astype(np.float16)
    bg = np.asarray(b_gate, dtype=np.float32).reshape(1, 1)
    in_maps = []
    for b in range(B):
        ctxT = context_hidden[b].T.astype(np.float16)  # [H, C]
        ctxb = np.ascontiguousarray(
            ctxT.reshape(JH, P, C).transpose(1, 0, 2))
        hidT = hidden_states[b].T.astype(np.float16)  # [H, G]
        hidb = np.ascontiguousarray(
            hidT.reshape(JH, P, G).transpose(1, 0, 2))
        maskc = np.ascontiguousarray(
            copy_mask[b].reshape(NCT, P).T.astype(np.int32))
        in_maps.append({
            "ctx": ctxb, "hid": hidb, "w": wcols, "mask": maskc, "bg": bg,
        })
    return in_maps


def unpack_out(res):
    outs = []
    for b in range(B):
        outb = np.asarray(res.results[b]["out"])  # [P, NCT, G] bf16
        outs.append(
            outb.transpose(2, 1, 0).reshape(G, C).astype(np.float32))
    return np.stack(outs, axis=0)


def kernel(hidden_states, context_hidden, encoder_output, w_attn, w_gate,
           b_gate, copy_mask):
    from concourse.bass_utils import run_bass_kernel_spmd

    nc = _get_nc()
    in_maps = make_in_maps(hidden_states, context_hidden, w_attn, w_gate,
                           b_gate, copy_mask)
    res = run_bass_kernel_spmd(nc, in_maps, core_ids=list(range(N_CORES)))
    return unpack_out(res)
